# revision 26
# baseline (speedup 1.0000x reference)
"""Column-sum kernel for Trainium2: out[d] = sum_r x[r, d].

x is [8192, 4096] f32, rows sharded across 8 NeuronCores (1024 rows
each). Per-core pipeline:

- All loads are gpsimd (SWDGE) DMAs that cast f32 -> bf16 inline, so
  the PE can reduce with single-pass bf16 ones-matmuls instead of the
  2-pass fp32 LOW_HIGH path, and SBUF writes halve.
- Rows 0..895 load as seven [128, 4096] row tiles. As tile k lands,
  eight [128, 512] chunk matmuls accumulate it into per-chunk PSUM
  banks (ones^T @ chunk). No DVE/GpSimd fold at all -- the PE does
  the whole reduction at ~215 ns per warm chunk matmul, hidden under
  the stream.
- Rows 896..1023 load as eight [128, 512] column stripes. Stripe c is
  the LAST data touching PSUM chunk c, so its close matmul, the ACT
  copy to SBUF, and its output slice fire as soon as it lands while
  later stripes still stream; only the last stripe's matmul+copy+
  store trails the final HBM byte (~2.5 us tail).
- DMA-completion semaphore lanes (8 total) force stripe k's
  descriptor emission to wait until tile (k-1)'s lane frees; those
  waiters are just that tile's 8 matmuls, which retire right after
  the tile lands, so emission stays ~5 us ahead of the stream. (A
  DMA whose lane waiters retire late -- e.g. a constants load read by
  every matmul -- stalls emission mid-stream for microseconds; keep
  such tensors out of DMA and memset them instead.)

Accumulation stays fp32 (PSUM); only the inputs round to bf16, so the
column-sum error is ~0.2% -- far inside the 2e-2 gate. Host sums the
8 per-core [1, 4096] f32 partials.
"""

import numpy as np

M_CORES = 8
ROWS, D = 8192, 4096
ROWS_PER_CORE = ROWS // M_CORES  # 1024
P = 128
NCHUNK = 512  # PSUM bank: fp32 [1, 512]; 8 chunks cover 4096 cols
NSTRIPE = D // NCHUNK
NTILE = 7  # row tiles, rows 0..895
COPY_GROUPS = ((0, 2), (2, 4), (4, 6), (6, 7), (7, 8))  # ACT copy batching
OUT_GROUPS = {3: (0, 4), 6: (4, 7), 7: (7, 8)}  # output DMA batching

_nc_cache = None


def _build():
    import concourse.tile as tile
    from concourse import bacc, mybir

    nc = bacc.Bacc(None)
    x = nc.declare_dram_parameter(
        "x", [ROWS_PER_CORE, D], mybir.dt.float32, isOutput=False
    )
    out = nc.declare_dram_parameter("out", [1, D], mybir.dt.float32, isOutput=True)

    with tile.TileContext(nc) as tc:
        with (
            tc.tile_pool(name="xpool", bufs=1) as xpool,
            tc.tile_pool(name="psum", bufs=1, space="PSUM") as psum_pool,
        ):
            ones = xpool.tile([P, 1], mybir.dt.bfloat16)
            nc.vector.memset(ones[:], 1.0)

            osb = xpool.tile([1, D], mybir.dt.float32)

            # 7 casting row-tile loads, then 8 casting stripe loads;
            # the single SWDGE queue keeps arrival in issue order.
            # Stripe k's issue reuses tile (k-1)'s completion-sem lane
            # (8 lanes total); those waiters are the tile's 8 matmuls,
            # which retire right after the tile lands, so descriptor
            # emission never stalls the stream.
            bts = []
            for k in range(NTILE):
                bt = xpool.tile([P, D], mybir.dt.bfloat16, name=f"bt{k}")
                nc.gpsimd.dma_start(bt[:], x[k * P : (k + 1) * P, :])
                bts.append(bt)
            sts = []
            for c in range(NSTRIPE):
                st = xpool.tile([P, NCHUNK], mybir.dt.bfloat16, name=f"st{c}")
                nc.gpsimd.dma_start(
                    st[:], x[NTILE * P :, c * NCHUNK : (c + 1) * NCHUNK]
                )
                sts.append(st)

            ps = psum_pool.tile([1, D], mybir.dt.float32, name="ps")

            # PE accumulates tile k into all 8 chunk banks as it lands.
            for k in range(NTILE):
                for c in range(NSTRIPE):
                    nc.tensor.matmul(
                        ps[:1, c * NCHUNK : (c + 1) * NCHUNK],
                        ones[:],
                        bts[k][:, c * NCHUNK : (c + 1) * NCHUNK],
                        start=(k == 0),
                        stop=False,
                    )

            # Stripe c closes chunk c the moment it lands; PSUM->SBUF
            # copies batch in pairs for chunks 0-5 (singles for 6 and
            # 7 so the final copy is short and never queued), and
            # outputs batch into three DMAs so the Sync queue is clear
            # when the last copy finishes.
            copy_after = {hi - 1: (lo, hi) for lo, hi in COPY_GROUPS}
            for c in range(NSTRIPE):
                nc.tensor.matmul(
                    ps[:1, c * NCHUNK : (c + 1) * NCHUNK],
                    ones[:],
                    sts[c][:],
                    start=False,
                    stop=True,
                )
                if c in copy_after:
                    lo, hi = copy_after[c]
                    nc.scalar.copy(
                        osb[:, lo * NCHUNK : hi * NCHUNK],
                        ps[:1, lo * NCHUNK : hi * NCHUNK],
                    )
                if c in OUT_GROUPS:
                    lo, hi = OUT_GROUPS[c]
                    nc.sync.dma_start(
                        out[:, lo * NCHUNK : hi * NCHUNK],
                        osb[:, lo * NCHUNK : hi * NCHUNK],
                    )

    nc.compile()
    return nc


def _get_nc():
    global _nc_cache
    if _nc_cache is None:
        _nc_cache = _build()
    return _nc_cache


def _run(x_np: np.ndarray, **run_kwargs):
    from concourse.bass_utils import run_bass_kernel_spmd

    nc = _get_nc()
    shards = np.split(x_np, M_CORES, axis=0)
    in_maps = [{"x": np.ascontiguousarray(s)} for s in shards]
    return run_bass_kernel_spmd(nc, in_maps, list(range(M_CORES)), **run_kwargs)


def kernel(x) -> np.ndarray:
    x_np = np.ascontiguousarray(np.asarray(x), dtype=np.float32)
    assert x_np.shape == (ROWS, D), x_np.shape
    res = _run(x_np)
    partials = np.stack([r["out"][0] for r in res.results])
    return partials.sum(axis=0, dtype=np.float32)
